# revision 1
# baseline (speedup 1.0000x reference)
"""Trainium2 Bass kernel for PointTactileTokenizer (retrieval_knn).

Contract: kernel(**inputs) takes the FULL unsharded inputs (numpy arrays, keys
as in setup_inputs) and returns the FULL output [B, 1+N+M, D] float32.

Strategy: data-parallel over batch B=8 across the 8 NeuronCores; one batch
element per core.  Per core:
  - point/tactile token MLPs in bf16 on the TensorEngine (feature-major)
  - kNN via a rank-score matmul  s[n,m] = [p,1]·[t,-|t|^2/2]  (f32r) so that
    the 8 largest scores == the 8 smallest distances; DVE Max/MaxIndex
    instructions produce exact per-row top-8 values+indices
  - token gather via GPSIMD dma_gather from an on-device row-major bf16 table
  - softmax(-d/T) weights; weighted sum via identity-lhsT matmuls into PSUM
Host does: positional-encoding concat/transposes, weight/bias prep (ctx_emb is
folded into biases / the gather table), final [D,T] -> [T,D] transposes.
"""

import numpy as np
import ml_dtypes

B, N, M, D = 8, 8192, 2048, 256
POINT_FEAT, TAC_FEAT = 6, 16
PE_BANDS, PE_MAX_FREQ = 6, 10.0
K_TACTILE, TAC_TEMP = 8, 0.05
IN_POINT = POINT_FEAT + 3 * 2 * PE_BANDS + 3 * 32   # 138
IN_TAC = TAC_FEAT + 3 * 2 * PE_BANDS                # 52
NT = N // 128                                        # 64 point tiles
BF16 = ml_dtypes.bfloat16

_NC_CACHE = {}


def _pe3_np(xyz):
    freqs = np.linspace(1.0, PE_MAX_FREQ, PE_BANDS, dtype=np.float32)
    x = xyz[..., None] * freqs * np.float32(np.pi)
    pe = np.concatenate([np.sin(x), np.cos(x)], axis=-1)
    return pe.reshape(xyz.shape[0], -1).astype(np.float32)


def _build_nc(repeat=None):
    import os
    if repeat is None:
        repeat = int(os.environ.get("KERNEL_REPEAT", "1"))
    import concourse.bass as bass
    import concourse.tile as tile
    from concourse import library_config
    from concourse import mybir
    from contextlib import ExitStack

    dt = mybir.dt
    AF = mybir.ActivationFunctionType
    ALU = mybir.AluOpType
    AX = mybir.AxisListType

    f32, bf, f32r, i16, u16 = dt.float32, dt.bfloat16, dt.float32r, dt.int16, dt.uint16

    nc = bass.Bass(num_swdge_queues=4)

    # ---- external inputs (per-core shard) ----
    pinA = nc.declare_dram_parameter("pinA", [128, N], bf, isOutput=False)
    pinB = nc.declare_dram_parameter("pinB", [IN_POINT - 128, N], bf, isOutput=False)
    tin = nc.declare_dram_parameter("tin", [IN_TAC, M], bf, isOutput=False)
    pt4 = nc.declare_dram_parameter("pt4", [4, N], f32r, isOutput=False)
    tt4 = nc.declare_dram_parameter("tt4", [4, M], f32r, isOutput=False)
    pnw = nc.declare_dram_parameter("pnw", [128, NT], f32, isOutput=False)
    Wp1a = nc.declare_dram_parameter("Wp1a", [128, D], bf, isOutput=False)
    Wp1b = nc.declare_dram_parameter("Wp1b", [IN_POINT - 128, D], bf, isOutput=False)
    Wp2 = nc.declare_dram_parameter("Wp2", [D, D], bf, isOutput=False)
    Wp3 = nc.declare_dram_parameter("Wp3", [D, D], bf, isOutput=False)
    Wt1 = nc.declare_dram_parameter("Wt1", [IN_TAC, D], bf, isOutput=False)
    Wt2 = nc.declare_dram_parameter("Wt2", [D, D], bf, isOutput=False)
    Wt3 = nc.declare_dram_parameter("Wt3", [D, D], bf, isOutput=False)
    # biases wrapped [128, 2]: chunk c of 128 at column c
    bp1w = nc.declare_dram_parameter("bp1w", [128, 2], f32, isOutput=False)
    bp2w = nc.declare_dram_parameter("bp2w", [128, 2], f32, isOutput=False)
    bt1w = nc.declare_dram_parameter("bt1w", [128, 2], f32, isOutput=False)
    bt2w = nc.declare_dram_parameter("bt2w", [128, 2], f32, isOutput=False)
    btow = nc.declare_dram_parameter("btow", [128, 2], f32, isOutput=False)   # bt3+ctx
    btabw = nc.declare_dram_parameter("btabw", [128, 2], f32, isOutput=False)  # bt3+ctx+bp3
    ident = nc.declare_dram_parameter("ident", [128, 128], bf, isOutput=False)

    out = nc.declare_dram_parameter("out", [D, N + M], f32, isOutput=True)

    # ---- internal DRAM ----
    ttok_rm = nc.dram_tensor("ttok_rm", [M, D], bf)        # gather table (row major)
    idxd = nc.dram_tensor("idxd", [NT, 128, 8], i16)       # idx bounce buffer
    idxw = nc.dram_tensor("idxw", [NT, 1024], i16)        # wrapped idx bounce

    with tile.TileContext(nc) as tc, ExitStack() as ctx:
        wpool = ctx.enter_context(tc.tile_pool(name="weights", bufs=1))
        hpool = ctx.enter_context(tc.tile_pool(name="acts", bufs=2))
        spool = ctx.enter_context(tc.tile_pool(name="scores", bufs=2))
        gpool = ctx.enter_context(tc.tile_pool(name="gath", bufs=2))
        ipool = ctx.enter_context(tc.tile_pool(name="idx", bufs=3))
        smol = ctx.enter_context(tc.tile_pool(name="small", bufs=4))
        opool = ctx.enter_context(tc.tile_pool(name="outs", bufs=4))
        tpool = ctx.enter_context(tc.tile_pool(name="ttok", bufs=1))
        ps_s = ctx.enter_context(tc.tile_pool(name="ps_s", bufs=2, space="PSUM"))
        ps_m = ctx.enter_context(tc.tile_pool(name="ps_m", bufs=2, space="PSUM"))
        ps_w = ctx.enter_context(tc.tile_pool(name="ps_w", bufs=1, space="PSUM"))
        ps_t = ctx.enter_context(tc.tile_pool(name="ps_t", bufs=2, space="PSUM"))

        nc.gpsimd.load_library(library_config.mlp)
        nidx_reg = nc.gpsimd.to_reg(1024)

        def load(pool, param, dtype=None, shape=None):
            t = pool.tile(shape or list(param.shape), dtype or param.dtype,
                          name=param.name + "_sb", tag=param.name + "_sb")
            nc.sync.dma_start(t[:], param[:])
            return t

        # ---- resident tiles ----
        ident_sb = load(wpool, ident)
        tin_sb = load(wpool, tin)
        wt1 = load(wpool, Wt1)

        def load2(param, name):
            ts = []
            for kc in range(2):
                t = wpool.tile([128, D], bf, tag=f"{name}{kc}", name=f"{name}{kc}")
                nc.sync.dma_start(t[:], param[kc * 128:(kc + 1) * 128, :])
                ts.append(t)
            return ts

        wt2 = load2(Wt2, "wt2")
        wt3 = load2(Wt3, "wt3")
        bt1 = load(wpool, bt1w)
        bt2 = load(wpool, bt2w)
        bto = load(wpool, btow)
        btab = load(wpool, btabw)

        # =============== Phase T: tactile tokens ===============
        h1t = [tpool.tile([128, M], bf, tag=f"h1t{d}", name=f"h1t{d}") for d in range(2)]
        h2t = [tpool.tile([128, M], bf, tag=f"h2t{d}", name=f"h2t{d}") for d in range(2)]
        ttok_out = [tpool.tile([128, M], f32, tag=f"tto{d}", name=f"tto{d}") for d in range(2)]
        tabf = [tpool.tile([128, M], bf, tag=f"tab{d}", name=f"tab{d}") for d in range(2)]

        for q in range(M // 512):
            sl = slice(q * 512, (q + 1) * 512)
            for dc in range(2):
                ps = ps_m.tile([128, 512], f32)
                nc.tensor.matmul(ps[:], wt1[:, dc * 128:(dc + 1) * 128], tin_sb[:, sl],
                                 start=True, stop=True)
                nc.scalar.activation(h1t[dc][:, sl], ps[:], AF.Gelu,
                                     bias=bt1[:, dc:dc + 1], scale=1.0)
        for q in range(M // 512):
            sl = slice(q * 512, (q + 1) * 512)
            for dc in range(2):
                ps = ps_m.tile([128, 512], f32)
                for kc in range(2):
                    nc.tensor.matmul(ps[:], wt2[kc][:, dc * 128:(dc + 1) * 128],
                                     h1t[kc][:, sl], start=(kc == 0), stop=(kc == 1))
                nc.scalar.activation(h2t[dc][:, sl], ps[:], AF.Gelu,
                                     bias=bt2[:, dc:dc + 1], scale=1.0)
        for q in range(M // 512):
            sl = slice(q * 512, (q + 1) * 512)
            for dc in range(2):
                ps = ps_m.tile([128, 512], f32)
                for kc in range(2):
                    nc.tensor.matmul(ps[:], wt3[kc][:, dc * 128:(dc + 1) * 128],
                                     h2t[kc][:, sl], start=(kc == 0), stop=(kc == 1))
                # output rows: ttok + bt3 + ctx (f32); table: + bp3 as well (bf16)
                nc.vector.tensor_scalar(ttok_out[dc][:, sl], ps[:], bto[:, dc:dc + 1], None, ALU.add)
                nc.vector.tensor_scalar(tabf[dc][:, sl], ps[:], btab[:, dc:dc + 1], None, ALU.add)

        for dc in range(2):
            nc.sync.dma_start(out[dc * 128:(dc + 1) * 128, N:N + M], ttok_out[dc][:])

        # gather table: transpose [feat, tok] -> ttok_rm [tok, feat] (bf16)
        for mc in range(M // 128):
            for dc in range(2):
                pst = ps_t.tile([128, 128], bf)
                nc.tensor.transpose(pst[:], tabf[dc][:, mc * 128:(mc + 1) * 128], ident_sb[:])
                stg = opool.tile([128, 128], bf, tag="tabstg")
                nc.scalar.activation(stg[:], pst[:], AF.Copy)
                nc.sync.dma_start(ttok_rm[mc * 128:(mc + 1) * 128, dc * 128:(dc + 1) * 128], stg[:])

        # =============== Phase P: points ===============
        pinA_sb = load(wpool, pinA)
        pinB_sb = load(wpool, pinB)
        pt4_sb = load(wpool, pt4)
        tt4_sb = load(wpool, tt4)
        pn_sb = load(wpool, pnw)
        wp1a = load(wpool, Wp1a)
        wp1b = load(wpool, Wp1b)
        wp2 = load2(Wp2, "wp2")
        wp3 = load2(Wp3, "wp3")
        bp1 = load(wpool, bp1w)
        bp2 = load(wpool, bp2w)

        for rep_ch in range(repeat * (N // 512)):
            ch = rep_ch % (N // 512)
            csl = slice(ch * 512, (ch + 1) * 512)
            h1p = [hpool.tile([128, 512], bf, tag=f"h1p{d}", name=f"h1p{d}") for d in range(2)]
            for dc in range(2):
                ps = ps_m.tile([128, 512], f32)
                nc.tensor.matmul(ps[:], wp1a[:, dc * 128:(dc + 1) * 128], pinA_sb[:, csl],
                                 start=True, stop=False)
                nc.tensor.matmul(ps[:], wp1b[:, dc * 128:(dc + 1) * 128], pinB_sb[:, csl],
                                 start=False, stop=True)
                nc.scalar.activation(h1p[dc][:], ps[:], AF.Gelu, bias=bp1[:, dc:dc + 1], scale=1.0)
            h2p = [hpool.tile([128, 512], bf, tag=f"h2p{d}", name=f"h2p{d}") for d in range(2)]
            for dc in range(2):
                ps = ps_m.tile([128, 512], f32)
                for kc in range(2):
                    nc.tensor.matmul(ps[:], wp2[kc][:, dc * 128:(dc + 1) * 128],
                                     h1p[kc][:], start=(kc == 0), stop=(kc == 1))
                nc.scalar.activation(h2p[dc][:], ps[:], AF.Gelu, bias=bp2[:, dc:dc + 1], scale=1.0)

            for sub in range(4):
                t = ch * 4 + sub
                tsl = slice(t * 128, (t + 1) * 128)
                ssl = slice(sub * 128, (sub + 1) * 128)

                # ---- scores: [128 pts, M] ----
                s_sb = spool.tile([128, M], f32, tag="s_sb")
                for q in range(M // 512):
                    ps = ps_s.tile([128, 512], f32)
                    nc.tensor.matmul(ps[:], pt4_sb[:, tsl], tt4_sb[:, q * 512:(q + 1) * 512],
                                     start=True, stop=True)
                    nc.scalar.activation(s_sb[:, q * 512:(q + 1) * 512], ps[:], AF.Copy)

                # ---- top-8 (largest score == nearest) ----
                v8 = smol.tile([128, 8], f32, tag="v8")
                nc.vector.max(v8[:], s_sb[:])
                i8 = smol.tile([128, 8], u16, tag="i8")
                nc.vector.max_index(i8[:], v8[:], s_sb[:])

                # ---- softmax(-d/T) weights ----
                d2 = smol.tile([128, 8], f32, tag="d2")
                nc.vector.tensor_scalar(d2[:], v8[:], -2.0, pn_sb[:, t:t + 1], ALU.mult, ALU.add)
                nc.vector.tensor_scalar_max(d2[:], d2[:], 0.0)
                dd = smol.tile([128, 8], f32, tag="dd")
                nc.scalar.activation(dd[:], d2[:], AF.Sqrt)
                uu = smol.tile([128, 8], f32, tag="uu")
                nc.vector.tensor_scalar(uu[:], dd[:], dd[:, 0:1], None, ALU.subtract)
                ee = smol.tile([128, 8], f32, tag="ee")
                nc.scalar.activation(ee[:], uu[:], AF.Exp, scale=-1.0 / TAC_TEMP)
                zz = smol.tile([128, 1], f32, tag="zz")
                nc.vector.reduce_sum(zz[:], ee[:], axis=AX.X)
                rz = smol.tile([128, 1], f32, tag="rz")
                nc.vector.reciprocal(rz[:], zz[:])
                ww = smol.tile([128, 8], f32, tag="ww")
                nc.vector.tensor_scalar(ww[:], ee[:], rz[:, 0:1], None, ALU.mult)

                # ---- indices -> wrapped gather layout ----
                nc.sync.dma_start(idxd[t], i8[:].bitcast(i16))
                idx_sb = ipool.tile([128, 64], i16, tag="idx")
                tmpi = ipool.tile([128, 8], i16, tag="tmpi")
                nc.sync.dma_start_transpose(tmpi[:], idxd[t].flatten().rearrange("(b c) -> b c", b=8))
                nc.sync.dma_start(idxw[t].rearrange("(p j) -> p j", p=128), tmpi[:])
                rep = idxw[t]
                rep = bass.AP(tensor=rep.tensor, offset=rep.offset,
                              ap=[[0, 8]] + list(rep.ap))
                nc.sync.dma_start(idx_sb[:], rep)

                # ---- gather tokens [128, 8, 256] bf16 ----
                G = gpool.tile([128, 8, D], bf, tag="G")
                nc.gpsimd.dma_gather(G[:], ttok_rm[:, :], idx_sb[:], num_idxs=1024,
                                     num_idxs_reg=nidx_reg, elem_size=D,
                                     queue_num=t % 4)

                # ---- weighted sum via identity matmuls ----
                Gw = gpool.tile([128, 8, D], bf, tag="Gw")
                for c in range(8):
                    nc.scalar.activation(Gw[:, c, :], G[:, c, :], AF.Copy, scale=ww[:, c:c + 1])
                ps_r = ps_w.tile([128, D], f32)
                for c in range(8):
                    nc.tensor.matmul(ps_r[:], ident_sb[:], Gw[:, c, :],
                                     start=(c == 0), stop=(c == 7))
                treg = opool.tile([128, D], bf, tag="treg")
                nc.scalar.activation(treg[:], ps_r[:], AF.Copy)

                # ---- point L3 + treg^T -> output ----
                for dc in range(2):
                    ps3 = ps_m.tile([128, 128], f32, tag="ps")
                    for kc in range(2):
                        nc.tensor.matmul(ps3[:], wp3[kc][:, dc * 128:(dc + 1) * 128],
                                         h2p[kc][:, ssl], start=(kc == 0), stop=(kc == 1))
                    pst = ps_t.tile([128, 128], bf)
                    nc.tensor.transpose(pst[:], treg[:, dc * 128:(dc + 1) * 128], ident_sb[:])
                    tt_sb = opool.tile([128, 128], f32, tag="tt_sb")
                    nc.scalar.activation(tt_sb[:], pst[:], AF.Copy)
                    o_sb = opool.tile([128, 128], f32, tag="o_sb")
                    nc.vector.tensor_tensor(o_sb[:], ps3[:], tt_sb[:], ALU.add)
                    nc.sync.dma_start(out[dc * 128:(dc + 1) * 128, tsl], o_sb[:])

    _split_sync_waits(nc)
    from concourse.library_overlay import lower_extended_insts
    lower_extended_insts(nc)
    return nc


def _split_sync_waits(nc, maxw=1):
    """This walrus build rejects instructions carrying several sem-waits
    ("Too many sync wait commands").  Hoist excess waits onto standalone
    event-semaphore instructions just before the carrier."""
    from concourse import mybir
    k = 0
    for f in nc.m.functions:
        for bb in f.blocks:
            insts = list(bb.instructions)
            out = []
            changed = False
            for inst in insts:
                si = inst.sync_info
                waits = list(si.on_wait) if si is not None and si.on_wait else []
                if len(waits) > maxw:
                    for w in waits[:-maxw]:
                        k += 1
                        ev = mybir.InstEventSemaphore(name=f"wsplit_{k}", ins=[], outs=[])
                        ev.engine = inst.engine
                        ev.sync_info = mybir.SyncInfo(on_wait=[w], on_update=[])
                        out.append(ev)
                    si.on_wait = waits[-maxw:]
                    changed = True
                out.append(inst)
            if changed:
                bb.instructions = out


def _host_prep(inputs):
    """Build per-core input maps from the full inputs."""
    f32 = np.float32
    p_xyz = np.asarray(inputs["point_xyz_norm"], f32)
    p_feat = np.asarray(inputs["point_feats"], f32)
    t_xyz = np.asarray(inputs["tactile_xyz_norm"], f32)
    t_feat = np.asarray(inputs["tactile_feats"], f32)
    tri = np.asarray(inputs["triplane_feats_at_points"], f32)
    ctx = np.asarray(inputs["ctx_emb"], f32)
    W = {k: np.asarray(inputs[k], f32) for k in
         ("Wp1", "bp1", "Wp2", "bp2", "Wp3", "bp3", "Wt1", "bt1", "Wt2", "bt2", "Wt3", "bt3")}

    def wrap_bias(v):  # [256] -> [128, 2]
        return np.ascontiguousarray(v.reshape(2, 128).T)

    ident = np.eye(128, dtype=BF16)
    in_maps = []
    for b in range(B):
        pe_p = _pe3_np(p_xyz[b])                      # [N, 36]
        point_in = np.concatenate([p_feat[b], pe_p, tri[b]], axis=1)   # [N, 138]
        pin_T = np.ascontiguousarray(point_in.T)      # [138, N]
        pe_t = _pe3_np(t_xyz[b])
        tac_in = np.ascontiguousarray(
            np.concatenate([t_feat[b], pe_t], axis=1).T)               # [52, M]

        pt4 = np.concatenate([p_xyz[b].T, np.ones((1, N), f32)], 0)    # [4, N]
        tt4 = np.concatenate([t_xyz[b].T,
                              -0.5 * np.sum(t_xyz[b] ** 2, 1)[None, :]], 0)  # [4, M]
        pn = np.sum(p_xyz[b] ** 2, 1).reshape(NT, 128).T               # [128, NT]

        m = {
            "pinA": pin_T[:128].astype(BF16),
            "pinB": np.ascontiguousarray(pin_T[128:]).astype(BF16),
            "tin": tac_in.astype(BF16),
            "pt4": np.ascontiguousarray(pt4),
            "tt4": np.ascontiguousarray(tt4),
            "pnw": np.ascontiguousarray(pn),
            "Wp1a": W["Wp1"][:128].astype(BF16),
            "Wp1b": np.ascontiguousarray(W["Wp1"][128:]).astype(BF16),
            "Wp2": W["Wp2"].astype(BF16),
            "Wp3": W["Wp3"].astype(BF16),
            "Wt1": W["Wt1"].astype(BF16),
            "Wt2": W["Wt2"].astype(BF16),
            "Wt3": W["Wt3"].astype(BF16),
            "bp1w": wrap_bias(W["bp1"]),
            "bp2w": wrap_bias(W["bp2"]),
            "bt1w": wrap_bias(W["bt1"]),
            "bt2w": wrap_bias(W["bt2"]),
            "btow": wrap_bias(W["bt3"] + ctx[b]),
            "btabw": wrap_bias(W["bt3"] + ctx[b] + W["bp3"]),
            "ident": ident,
        }
        in_maps.append(m)
    return in_maps


def kernel(**inputs):
    from concourse.bass_utils import run_bass_kernel_spmd

    if "nc" not in _NC_CACHE:
        _NC_CACHE["nc"] = _build_nc()
    nc = _NC_CACHE["nc"]

    import os
    in_maps = _host_prep(inputs)
    trace = bool(int(os.environ.get("KERNEL_TRACE", "0")))
    res = run_bass_kernel_spmd(nc, in_maps, core_ids=list(range(B)), trace=trace)
    _NC_CACHE["last_result"] = res

    ctx = np.asarray(inputs["ctx_emb"], np.float32)
    gtok = np.asarray(inputs["global_token"], np.float32).reshape(D)
    out = np.empty((B, 1 + N + M, D), np.float32)
    for b in range(B):
        fm = np.asarray(res.results[b]["out"])       # [D, N+M]
        out[b, 0] = gtok + ctx[b]
        out[b, 1:] = fm.T
    return out


def benchmark(inputs, iters=20):
    """Time repeated on-device executions (inputs pre-staged, no donation)."""
    import time
    import jax
    import jax.numpy as jnp
    from jax.sharding import Mesh, PartitionSpec
    from jax.experimental.shard_map import shard_map
    from concourse import bass2jax as b2j

    if "nc" not in _NC_CACHE:
        _NC_CACHE["nc"] = _build_nc()
    nc = _NC_CACHE["nc"]
    b2j.install_neuronx_cc_hook()

    in_maps = _host_prep(inputs)
    from concourse import mybir
    in_names, out_names, out_avals, zero_outs = [], [], [], []
    partition_name = nc.partition_id_tensor.name if nc.partition_id_tensor else None
    for alloc in nc.m.functions[0].allocations:
        if not isinstance(alloc, mybir.MemoryLocationSet):
            continue
        name = alloc.memorylocations[0].name
        if alloc.kind == "ExternalInput":
            if name != partition_name:
                in_names.append(name)
        elif alloc.kind == "ExternalOutput":
            out_names.append(name)
            shape = list(alloc.tensor_shape)
            np_dt = np.dtype(mybir.dt.np(alloc.dtype))
            out_avals.append(jax.core.ShapedArray(shape, np_dt))
            zero_outs.append(np.zeros(shape, np_dt))
    n_params = len(in_names)
    all_in_names = list(in_names) + out_names
    if partition_name is not None:
        all_in_names.append(partition_name)

    def _body(*args):
        operands = list(args)
        if partition_name is not None:
            operands.append(b2j.partition_id_tensor())
        outs = b2j._bass_exec_p.bind(
            *operands, out_avals=tuple(out_avals), in_names=tuple(all_in_names),
            out_names=tuple(out_names), lowering_input_output_aliases=(),
            sim_require_finite=True, sim_require_nnan=True, nc=nc)
        return tuple(outs)

    devices = jax.devices()[:B]
    mesh = Mesh(np.asarray(devices), ("core",))
    nio = n_params + len(out_names)
    fn = jax.jit(shard_map(_body, mesh=mesh,
                           in_specs=(PartitionSpec("core"),) * nio,
                           out_specs=(PartitionSpec("core"),) * len(out_names),
                           check_rep=False), keep_unused=True)
    concat_in = [np.concatenate([np.asarray(in_maps[c][n]) for c in range(B)], axis=0)
                 for n in in_names]
    concat_zeros = [np.zeros((B * z.shape[0], *z.shape[1:]), z.dtype) for z in zero_outs]
    from jax.sharding import NamedSharding
    sh = NamedSharding(mesh, PartitionSpec("core"))
    dev_in = [jax.device_put(x, sh) for x in concat_in + concat_zeros]
    _NC_CACHE["bench_fn"] = (fn, dev_in)
    outs = fn(*dev_in)
    jax.block_until_ready(outs)
    times = []
    for _ in range(iters):
        t0 = time.perf_counter()
        outs = fn(*dev_in)
        jax.block_until_ready(outs)
        times.append(time.perf_counter() - t0)
    return min(times), times


def benchmark_pipelined(inputs, n_lo=100, n_hi=200):
    """Marginal per-call time from pipelined async dispatches: amortizes the
    axon round-trip latency; returns (T(n_hi)-T(n_lo))/(n_hi-n_lo) seconds."""
    import time
    import jax
    # benchmark() must have been called first (compiles + stages buffers)
    best, _ = benchmark(inputs, iters=1)
    fn, dev_in = _NC_CACHE["bench_fn"]
    out = None
    ts = {}
    for n in (n_lo, n_hi):
        t0 = time.perf_counter()
        outs = [fn(*dev_in) for _ in range(n)]
        jax.block_until_ready(outs)
        ts[n] = time.perf_counter() - t0
    return (ts[n_hi] - ts[n_lo]) / (n_hi - n_lo)



# revision 4
# speedup vs baseline: 7.3444x; 7.3444x over previous
"""Trainium2 Bass kernel for PointTactileTokenizer (retrieval_knn) — v2.

Contract: kernel(**inputs) takes the FULL unsharded inputs (numpy arrays, keys
as in setup_inputs) and returns the FULL output [B, 1+N+M, D] float32.

Strategy: data-parallel over batch B=8 across the 8 NeuronCores; one batch
element per core.  Per core:
  - point/tactile token MLPs in bf16 on the TensorEngine (feature-major)
  - kNN scores r[n,m] = p.t - |t|^2/2 - |p|^2/2 = -d^2/2 via one f32r matmul
    per 128-point tile (the -|p|^2/2 bias folds into the PSUM->SBUF copy);
    DVE Max/MaxIndex give exact per-row top-8 values + indices
  - softmax(-d/T) weights batched per 512-point chunk; the 32 weights are
    skew-DMA'd onto the diagonals of a pre-zeroed DRAM scratch and read back
    as dense [128, 32x128] diagonal matrices (zero engine cost)
  - tokens gathered per chunk (4096 idx) via GPSIMD dma_gather from a
    row-major bf16 table; weighted sum as 16 matmuls per tile with the
    gathered tokens as lhsT and the diagonal weights as rhs -- the result
    lands feature-major in the same PSUM tile where the point-MLP L3
    accumulates, so one activation copy emits the final output rows.
Host does: positional-encoding concat/transposes, weight/bias prep (ctx_emb
and bp3 fold into the gather table; bt3+ctx into the tactile rows), final
[D,T] -> [T,D] transpose and bf16->f32 upcast.
"""

import numpy as np
import ml_dtypes

B, N, M, D = 8, 8192, 2048, 256
POINT_FEAT, TAC_FEAT = 6, 16
PE_BANDS, PE_MAX_FREQ = 6, 10.0
K_TACTILE, TAC_TEMP = 8, 0.05
IN_POINT = POINT_FEAT + 3 * 2 * PE_BANDS + 3 * 32   # 138
IN_TAC = TAC_FEAT + 3 * 2 * PE_BANDS                # 52
NT = N // 128                                        # 64 point tiles
NCH = N // 512                                       # 16 chunks
BF16 = ml_dtypes.bfloat16

_NC_CACHE = {}

# blob packing specs: (name, rows, cols); order defines offsets
BF_SPECS = [
    ("pinA", 128, N), ("pinB", IN_POINT - 128, N), ("tin", IN_TAC, M),
    ("Wp1a", 128, D), ("Wp1b", IN_POINT - 128, D), ("Wp2", D, D), ("Wp3", D, D),
    ("Wt1", IN_TAC, D), ("Wt2", D, D), ("Wt3", D, D), ("ident", 128, 128),
]
F32_SPECS = [
    ("pt5", 5, N), ("tt5", 5, M),
    ("bp1w", 128, 2), ("bp2w", 128, 2), ("bt1w", 128, 2), ("bt2w", 128, 2),
    ("btow", 128, 2), ("btabw", 128, 2),
]


def _blob_offsets():
    boff, foff = {}, {}
    o = 0
    for nm, p, c in BF_SPECS:
        boff[nm] = o
        o += p * c
    o = 0
    for nm, p, c in F32_SPECS:
        foff[nm] = o
        o += p * c
    return boff, foff




def _pe3_np(xyz):
    freqs = np.linspace(1.0, PE_MAX_FREQ, PE_BANDS, dtype=np.float32)
    x = xyz[..., None] * freqs * np.float32(np.pi)
    pe = np.concatenate([np.sin(x), np.cos(x)], axis=-1)
    return pe.reshape(xyz.shape[0], -1).astype(np.float32)


def _build_nc(repeat=None):
    import os
    import concourse.bass as bass
    import concourse.bass as _bass
    import concourse.tile as tile
    from concourse import library_config
    from concourse import mybir
    from contextlib import ExitStack

    dt = mybir.dt
    AF = mybir.ActivationFunctionType
    ALU = mybir.AluOpType
    AX = mybir.AxisListType

    f32, bf, f32r, i16, u16 = dt.float32, dt.bfloat16, dt.float32r, dt.int16, dt.uint16

    nc = bass.Bass(num_swdge_queues=4)

    # ---- external inputs packed into two blobs (fewer per-call args) ----
    bf_tot = sum(p * c for _, p, c in BF_SPECS)
    f32_tot = sum(p * c for _, p, c in F32_SPECS)
    blob_bf = nc.declare_dram_parameter("blob_bf", [bf_tot], bf, isOutput=False)
    blob_f32 = nc.declare_dram_parameter("blob_f32", [f32_tot], f32, isOutput=False)
    BOFF, FOFF = _blob_offsets()

    class BlobView:
        """Mimics a 2D dram parameter as a view into a flat blob."""
        def __init__(self, blob, off, p, c, dtype, name):
            self.blob, self.off, self.p, self.c = blob, off, p, c
            self.dtype, self.name, self.shape = dtype, name, [p, c]

        def _ap(self, rows, cols):
            import concourse.bass as _b
            r0, r1 = rows.indices(self.p)[0], rows.indices(self.p)[1]
            c0, c1 = cols.indices(self.c)[0], cols.indices(self.c)[1]
            base = self.blob[:]
            return _b.AP(tensor=base.tensor,
                         offset=base.offset + self.off + r0 * self.c + c0,
                         ap=[[self.c, r1 - r0], [1, c1 - c0]])

        def __getitem__(self, key):
            if key == slice(None):
                return self._ap(slice(None), slice(None))
            rows, cols = key
            return self._ap(rows, cols)

    V = {}
    for nm, p, c in BF_SPECS:
        V[nm] = BlobView(blob_bf, BOFF[nm], p, c, bf, nm)
    for nm, p, c in F32_SPECS:
        V[nm] = BlobView(blob_f32, FOFF[nm], p, c, f32, nm)
    pinA, pinB, tin = V["pinA"], V["pinB"], V["tin"]
    Wp1a, Wp1b, Wp2, Wp3 = V["Wp1a"], V["Wp1b"], V["Wp2"], V["Wp3"]
    Wt1, Wt2, Wt3, ident = V["Wt1"], V["Wt2"], V["Wt3"], V["ident"]
    pt5, tt5 = V["pt5"], V["tt5"]
    bp1w, bp2w, bt1w, bt2w = V["bp1w"], V["bp2w"], V["bt1w"], V["bt2w"]
    btow, btabw = V["btow"], V["btabw"]

    out = nc.declare_dram_parameter("out", [D, N + M], bf, isOutput=True)

    # ---- internal DRAM ----
    ttok_rm = nc.dram_tensor("ttok_rm", [M, D], bf)        # gather table (row major)
    idxd = nc.dram_tensor("idxd", [NT, 128, 8], i16)       # idx bounce buffer
    idxw = nc.dram_tensor("idxw", [NT, 1024], i16)         # wrapped idx lists (per tile)

    with tile.TileContext(nc) as tc, ExitStack() as ctx:
        wpool = ctx.enter_context(tc.tile_pool(name="weights", bufs=1))
        hpool = ctx.enter_context(tc.tile_pool(name="acts", bufs=2))
        spool = ctx.enter_context(tc.tile_pool(name="scores", bufs=2))
        gpool = ctx.enter_context(tc.tile_pool(name="gath", bufs=2))
        dpool = ctx.enter_context(tc.tile_pool(name="diag", bufs=2))
        ipool = ctx.enter_context(tc.tile_pool(name="idx", bufs=2))
        i8pool = ctx.enter_context(tc.tile_pool(name="i8", bufs=4))
        smol = ctx.enter_context(tc.tile_pool(name="small", bufs=2))
        opool = ctx.enter_context(tc.tile_pool(name="outs", bufs=4))
        tpool = ctx.enter_context(tc.tile_pool(name="ttok", bufs=1))
        ppool = ctx.enter_context(tc.tile_pool(name="pt4c", bufs=2))
        ps_s = ctx.enter_context(tc.tile_pool(name="ps_s", bufs=1, space="PSUM"))
        ps_m = ctx.enter_context(tc.tile_pool(name="ps_m", bufs=2, space="PSUM"))
        ps_o = ctx.enter_context(tc.tile_pool(name="ps_o", bufs=2, space="PSUM"))

        nc.gpsimd.load_library(library_config.mlp)
        nidx_reg = nc.gpsimd.to_reg(1024)

        def load(pool, param, dtype=None, shape=None):
            t = pool.tile(shape or list(param.shape), dtype or param.dtype,
                          name=param.name + "_sb", tag=param.name + "_sb")
            srcap = param[:]
            if dtype is not None and dtype != param.dtype:
                srcap = srcap.bitcast(dtype)
            nc.sync.dma_start(t[:], srcap)
            return t

        # ---- resident tiles (point-phase loads first: chunk 0 starts early) ----
        ident_sb = load(wpool, ident)

        def load2(param, name):
            ts = []
            for kc in range(2):
                t = wpool.tile([128, D], bf, tag=f"{name}{kc}", name=f"{name}{kc}")
                nc.sync.dma_start(t[:], param[kc * 128:(kc + 1) * 128, :])
                ts.append(t)
            return ts

        wp1a = load(wpool, Wp1a)
        wp1b = load(wpool, Wp1b)
        bp1 = load(wpool, bp1w)
        bp2 = load(wpool, bp2w)
        tt5_sb = load(wpool, tt5)
        wp2 = load2(Wp2, "wp2")
        wp3 = load2(Wp3, "wp3")
        tin_sb = load(wpool, tin)
        wt1 = load(wpool, Wt1)
        wt2 = load2(Wt2, "wt2")
        wt3 = load2(Wt3, "wt3")
        bt1 = load(wpool, bt1w)
        bt2 = load(wpool, bt2w)
        bto = load(wpool, btow)
        btab = load(wpool, btabw)

        # =============== Phase T: tactile tokens ===============
        h1t = [tpool.tile([128, M], bf, tag=f"h1t{d}", name=f"h1t{d}") for d in range(2)]
        h2t = [tpool.tile([128, M], bf, tag=f"h2t{d}", name=f"h2t{d}") for d in range(2)]
        ttok_out = [tpool.tile([128, M], bf, tag=f"tto{d}", name=f"tto{d}") for d in range(2)]
        tabf = [tpool.tile([128, M], bf, tag=f"tab{d}", name=f"tab{d}") for d in range(2)]

        for q in range(M // 512):
            sl = slice(q * 512, (q + 1) * 512)
            for dc in range(2):
                ps = ps_m.tile([128, 512], f32)
                nc.tensor.matmul(ps[:], wt1[:, dc * 128:(dc + 1) * 128], tin_sb[:, sl],
                                 start=True, stop=True)
                nc.scalar.activation(h1t[dc][:, sl], ps[:], AF.Gelu,
                                     bias=bt1[:, dc:dc + 1], scale=1.0)
        for q in range(M // 512):
            sl = slice(q * 512, (q + 1) * 512)
            for dc in range(2):
                ps = ps_m.tile([128, 512], f32)
                for kc in range(2):
                    nc.tensor.matmul(ps[:], wt2[kc][:, dc * 128:(dc + 1) * 128],
                                     h1t[kc][:, sl], start=(kc == 0), stop=(kc == 1))
                nc.scalar.activation(h2t[dc][:, sl], ps[:], AF.Gelu,
                                     bias=bt2[:, dc:dc + 1], scale=1.0)
        for q in range(M // 512):
            sl = slice(q * 512, (q + 1) * 512)
            for dc in range(2):
                ps = ps_m.tile([128, 512], f32)
                for kc in range(2):
                    nc.tensor.matmul(ps[:], wt3[kc][:, dc * 128:(dc + 1) * 128],
                                     h2t[kc][:, sl], start=(kc == 0), stop=(kc == 1))
                # output rows: ttok + bt3 + ctx; table: + bp3 as well
                nc.scalar.activation(ttok_out[dc][:, sl], ps[:], AF.Identity,
                                     bias=bto[:, dc:dc + 1], scale=1.0)
                nc.scalar.activation(tabf[dc][:, sl], ps[:], AF.Identity,
                                     bias=btab[:, dc:dc + 1], scale=1.0)

        for dc in range(2):
            nc.sync.dma_start(out[dc * 128:(dc + 1) * 128, N:N + M], ttok_out[dc][:])

        # gather table: transpose [feat, tok] -> ttok_rm [tok, feat] (bf16)
        for mc in range(M // 128):
            pst = ps_o.tile([128, 256], bf, tag="ps_ot", name="pst")
            for dc in range(2):
                nc.tensor.transpose(pst[:, dc * 128:(dc + 1) * 128],
                                    tabf[dc][:, mc * 128:(mc + 1) * 128], ident_sb[:])
            stg = opool.tile([128, 256], bf, tag="tabstg")
            nc.scalar.activation(stg[:], pst[:], AF.Copy)
            nc.sync.dma_start(ttok_rm[mc * 128:(mc + 1) * 128, :], stg[:])

        # =============== Phase P: points ===============

        if repeat is None:
            repeat = int(os.environ.get("KERNEL_REPEAT", "1"))

        for rep_ch in range(repeat * NCH):
            ch = rep_ch % NCH
            csl = slice(ch * 512, (ch + 1) * 512)
            # chunk slice of the f32r point matrix [4, 512]
            pt5c = ppool.tile([5, 512], f32r, tag="pt5c", name="pt5c")
            nc.sync.dma_start(pt5c[:], pt5[:, csl].bitcast(f32r))
            pAc = ppool.tile([128, 512], bf, tag="pAc", name="pAc")
            nc.sync.dma_start(pAc[:], pinA[:, csl])
            pBc = ppool.tile([IN_POINT - 128, 512], bf, tag="pBc", name="pBc")
            nc.sync.dma_start(pBc[:], pinB[:, csl])

            # ---- per-tile scores + top-8 ----
            v8 = smol.tile([128, 32], f32, tag="v8")
            for sub in range(4):
                t = ch * 4 + sub
                # r[n,m] = p.t - |t|^2/2  (PSUM f32), copied with bias -|p|^2/2
                ps_sc = ps_s.tile([128, 2048], f32)
                for q in range(4):
                    nc.tensor.matmul(ps_sc[:, q * 512:(q + 1) * 512],
                                     pt5c[:, sub * 128:(sub + 1) * 128],
                                     tt5_sb[:, q * 512:(q + 1) * 512],
                                     start=True, stop=True)
                s_sb = spool.tile([128, 2048], f32, tag="s_sb")
                nc.scalar.activation(s_sb[:], ps_sc[:], AF.Copy)
                nc.vector.max(v8[:, sub * 8:(sub + 1) * 8], s_sb[:])
                i8 = i8pool.tile([128, 8], u16, tag="i8")
                nc.vector.max_index(i8[:], v8[:, sub * 8:(sub + 1) * 8], s_sb[:])
                nc.sync.dma_start(idxd[t], i8[:].bitcast(i16))
                tmpi = ipool.tile([128, 8], i16, tag="tmpi")
                nc.sync.dma_start_transpose(tmpi[:], idxd[t].flatten().rearrange("(b c) -> b c", b=8))
                nc.sync.dma_start(idxw[t].rearrange("(p j) -> p j", p=128), tmpi[:])

            h1p = [hpool.tile([128, 512], bf, tag=f"h1p{d}", name=f"h1p{d}") for d in range(2)]
            for dc in range(2):
                ps = ps_m.tile([128, 512], f32)
                nc.tensor.matmul(ps[:], wp1a[:, dc * 128:(dc + 1) * 128], pAc[:],
                                 start=True, stop=False)
                nc.tensor.matmul(ps[:], wp1b[:, dc * 128:(dc + 1) * 128], pBc[:],
                                 start=False, stop=True)
                nc.scalar.activation(h1p[dc][:], ps[:], AF.Gelu, bias=bp1[:, dc:dc + 1], scale=1.0)
            h2p = [hpool.tile([128, 512], bf, tag=f"h2p{d}", name=f"h2p{d}") for d in range(2)]
            for dc in range(2):
                ps = ps_m.tile([128, 512], f32)
                for kc in range(2):
                    nc.tensor.matmul(ps[:], wp2[kc][:, dc * 128:(dc + 1) * 128],
                                     h1p[kc][:], start=(kc == 0), stop=(kc == 1))
                nc.scalar.activation(h2p[dc][:], ps[:], AF.Gelu, bias=bp2[:, dc:dc + 1], scale=1.0)

            # ---- batched softmax(-d/T) weights for the whole chunk [128, 32] ----
            import concourse.bass as _bass

            def bcast_ap(tile_ap, outer_stride, outer_count, inner_count):
                return _bass.AP(tensor=tile_ap.tensor, offset=tile_ap.offset,
                                ap=[list(tile_ap.ap[0])] +
                                   [[outer_stride, outer_count], [0, inner_count]])

            d2 = smol.tile([128, 32], f32, tag="d2")
            nc.vector.tensor_scalar(d2[:], v8[:], -2.0, None, ALU.mult)
            nc.vector.tensor_scalar_max(d2[:], d2[:], 0.0)
            dd = smol.tile([128, 32], f32, tag="dd")
            nc.scalar.activation(dd[:], d2[:], AF.Sqrt)
            du = smol.tile([128, 32], f32, tag="du")
            nc.vector.tensor_tensor(du[:].rearrange("p (g c) -> p g c", g=4), dd[:].rearrange("p (g c) -> p g c", g=4),
                                    bcast_ap(dd[:], 8, 4, 8), ALU.subtract)
            # exp(-du/T) = (1 - t)/(1 + t) with t = tanh(du/(2T)); tanh shares
            # the Gelu act-table set, saving one table reload per chunk
            th = smol.tile([128, 32], f32, tag="th")
            nc.scalar.activation(th[:], du[:], AF.Tanh, scale=0.5 / TAC_TEMP)
            ta = smol.tile([128, 32], f32, tag="ta")
            nc.vector.tensor_scalar(ta[:], th[:], -1.0, 1.0, ALU.mult, ALU.add)
            tb = smol.tile([128, 32], f32, tag="tb")
            nc.vector.tensor_scalar(tb[:], th[:], 1.0, None, ALU.add)
            rb = smol.tile([128, 32], f32, tag="rb")
            nc.vector.reciprocal(rb[:], tb[:])
            ee = smol.tile([128, 32], f32, tag="ee")
            nc.vector.tensor_tensor(ee[:], ta[:], rb[:], ALU.mult)
            zz = smol.tile([128, 4], f32, tag="zz")
            nc.vector.reduce_sum(zz[:], ee[:].rearrange("p (g c) -> p g c", g=4), axis=AX.X)
            rz = smol.tile([128, 4], f32, tag="rz")
            nc.vector.reciprocal(rz[:], zz[:])
            ww = smol.tile([128, 32], f32, tag="ww")
            nc.vector.tensor_tensor(ww[:].rearrange("p (g c) -> p g c", g=4), ee[:].rearrange("p (g c) -> p g c", g=4),
                                    bcast_ap(rz[:], 1, 4, 8), ALU.mult)

            # ---- expand the 32 weights into diagonal matrices (split Act/DVE) ----
            NACT = 15
            diag_sb = dpool.tile([128, 4096], bf, tag="diag")
            for c in range(NACT):
                nc.scalar.activation(diag_sb[:, c * 128:(c + 1) * 128], ident_sb[:],
                                     AF.Copy, scale=ww[:, c:c + 1])
            iap = ident_sb[:]
            nrem = 32 - NACT
            ident_rep = _bass.AP(tensor=iap.tensor, offset=iap.offset,
                                 ap=[list(iap.ap[0])] + [[0, nrem], [1, 128]])
            ww_rep = _bass.AP(tensor=ww[:].tensor, offset=ww[:].offset + NACT,
                              ap=[list(ww[:].ap[0])] + [[1, nrem], [0, 128]])
            dtail = diag_sb[:, NACT * 128:]
            nc.vector.tensor_tensor(dtail.rearrange("p (c j) -> p c j", c=nrem),
                                    ident_rep, ww_rep, ALU.mult)

            # ---- assemble wrapped idx list and gather 4096 tokens ----
            G = gpool.tile([128, 32, D], bf, tag="G")
            for sub in range(4):
                t = ch * 4 + sub
                wbase = idxw[t]
                wrep = _bass.AP(tensor=wbase.tensor, offset=wbase.offset,
                                ap=[[0, 8], [1, 1024]])
                idx_sb = ipool.tile([128, 64], i16, tag="idx")
                nc.sync.dma_start(idx_sb[:], wrep)
                nc.gpsimd.dma_gather(G[:, sub * 8:(sub + 1) * 8, :], ttok_rm[:, :],
                                     idx_sb[:], num_idxs=1024,
                                     num_idxs_reg=nidx_reg, elem_size=D,
                                     queue_num=t % 4)

            # ---- weighted sum (as lhsT) + point L3 into one PSUM tile ----
            for sub in range(4):
                t = ch * 4 + sub
                ssl = slice(sub * 128, (sub + 1) * 128)
                ps_ot = ps_o.tile([128, 256], f32, tag="ps_ot", name="ps_ot")
                for dc in range(2):
                    osl = slice(dc * 128, (dc + 1) * 128)
                    for c in range(8):
                        c32 = sub * 8 + c
                        nc.tensor.matmul(ps_ot[:, osl],
                                         G[:, c32, osl],
                                         diag_sb[:, c32 * 128:(c32 + 1) * 128],
                                         start=(c == 0), stop=False)
                    for kc in range(2):
                        nc.tensor.matmul(ps_ot[:, osl],
                                         wp3[kc][:, osl],
                                         h2p[kc][:, ssl],
                                         start=False, stop=(kc == 1))
                o_sb = opool.tile([128, 256], bf, tag="o_sb")
                nc.scalar.activation(o_sb[:], ps_ot[:], AF.Copy)
                obase = out[:]
                odst = _bass.AP(tensor=obase.tensor,
                                offset=obase.offset + t * 128,
                                ap=[[N + M, 128], [(N + M) * 128, 2], [1, 128]])
                nc.sync.dma_start(odst, o_sb[:])

    _split_sync_waits(nc)
    from concourse.library_overlay import lower_extended_insts
    lower_extended_insts(nc)
    return nc


def _split_sync_waits(nc, maxw=1):
    """This walrus build rejects instructions carrying several sem-waits
    ("Too many sync wait commands").  Hoist excess waits onto standalone
    event-semaphore instructions just before the carrier."""
    from concourse import mybir
    k = 0
    for f in nc.m.functions:
        for bb in f.blocks:
            insts = list(bb.instructions)
            out = []
            changed = False
            for inst in insts:
                si = inst.sync_info
                waits = list(si.on_wait) if si is not None and si.on_wait else []
                if len(waits) > maxw:
                    for w in waits[:-maxw]:
                        k += 1
                        ev = mybir.InstEventSemaphore(name=f"wsplit_{k}", ins=[], outs=[])
                        ev.engine = inst.engine
                        ev.sync_info = mybir.SyncInfo(on_wait=[w], on_update=[])
                        out.append(ev)
                    si.on_wait = waits[-maxw:]
                    changed = True
                out.append(inst)
            if changed:
                bb.instructions = out


def _host_prep(inputs):
    """Build per-core input maps from the full inputs."""
    f32 = np.float32
    p_xyz = np.asarray(inputs["point_xyz_norm"], f32)
    p_feat = np.asarray(inputs["point_feats"], f32)
    t_xyz = np.asarray(inputs["tactile_xyz_norm"], f32)
    t_feat = np.asarray(inputs["tactile_feats"], f32)
    tri = np.asarray(inputs["triplane_feats_at_points"], f32)
    ctx = np.asarray(inputs["ctx_emb"], f32)
    W = {k: np.asarray(inputs[k], f32) for k in
         ("Wp1", "bp1", "Wp2", "bp2", "Wp3", "bp3", "Wt1", "bt1", "Wt2", "bt2", "Wt3", "bt3")}

    def wrap_bias(v):  # [256] -> [128, 2]
        return np.ascontiguousarray(v.reshape(2, 128).T)

    ident = np.eye(128, dtype=BF16)
    zeros = np.zeros((128, 4096), dtype=BF16)
    in_maps = []
    for b in range(B):
        pe_p = _pe3_np(p_xyz[b])                      # [N, 36]
        point_in = np.concatenate([p_feat[b], pe_p, tri[b]], axis=1)   # [N, 138]
        pin_T = np.ascontiguousarray(point_in.T)      # [138, N]
        pe_t = _pe3_np(t_xyz[b])
        tac_in = np.ascontiguousarray(
            np.concatenate([t_feat[b], pe_t], axis=1).T)               # [52, M]

        pt5 = np.concatenate([p_xyz[b].T, np.ones((1, N), f32),
                              -0.5 * np.sum(p_xyz[b] ** 2, 1)[None, :]], 0)   # [5, N]
        tt5 = np.concatenate([t_xyz[b].T,
                              -0.5 * np.sum(t_xyz[b] ** 2, 1)[None, :],
                              np.ones((1, M), f32)], 0)                       # [5, M]

        vals = {
            "pinA": pin_T[:128].astype(BF16),
            "pinB": np.ascontiguousarray(pin_T[128:]).astype(BF16),
            "tin": tac_in.astype(BF16),
            "pt5": np.ascontiguousarray(pt5),
            "tt5": np.ascontiguousarray(tt5),
            "Wp1a": W["Wp1"][:128].astype(BF16),
            "Wp1b": np.ascontiguousarray(W["Wp1"][128:]).astype(BF16),
            "Wp2": W["Wp2"].astype(BF16),
            "Wp3": W["Wp3"].astype(BF16),
            "Wt1": W["Wt1"].astype(BF16),
            "Wt2": W["Wt2"].astype(BF16),
            "Wt3": W["Wt3"].astype(BF16),
            "bp1w": wrap_bias(W["bp1"]),
            "bp2w": wrap_bias(W["bp2"]),
            "bt1w": wrap_bias(W["bt1"]),
            "bt2w": wrap_bias(W["bt2"]),
            "btow": wrap_bias(W["bt3"] + ctx[b]),
            "btabw": wrap_bias(W["bt3"] + ctx[b] + W["bp3"]),
            "ident": ident,
        }
        blob_bf = np.concatenate([np.asarray(vals[nm], BF16).reshape(-1)
                                  for nm, _, _ in BF_SPECS])
        blob_f32 = np.concatenate([np.asarray(vals[nm], f32).reshape(-1)
                                   for nm, _, _ in F32_SPECS])
        in_maps.append({"blob_bf": blob_bf, "blob_f32": blob_f32})
    return in_maps


def kernel(**inputs):
    from concourse.bass_utils import run_bass_kernel_spmd

    if "nc" not in _NC_CACHE:
        _NC_CACHE["nc"] = _build_nc()
    nc = _NC_CACHE["nc"]

    import os
    in_maps = _host_prep(inputs)
    trace = bool(int(os.environ.get("KERNEL_TRACE", "0")))
    res = run_bass_kernel_spmd(nc, in_maps, core_ids=list(range(B)), trace=trace)
    _NC_CACHE["last_result"] = res

    ctx = np.asarray(inputs["ctx_emb"], np.float32)
    gtok = np.asarray(inputs["global_token"], np.float32).reshape(D)
    out = np.empty((B, 1 + N + M, D), np.float32)
    for b in range(B):
        fm = np.asarray(res.results[b]["out"]).astype(np.float32)   # [D, N+M]
        out[b, 0] = gtok + ctx[b]
        out[b, 1:] = fm.T
    return out


def benchmark(inputs, iters=20, repeat=1):
    """Time repeated on-device executions (inputs pre-staged, no donation)."""
    import time
    import jax
    import jax.numpy as jnp
    from jax.sharding import Mesh, PartitionSpec
    from jax.experimental.shard_map import shard_map
    from concourse import bass2jax as b2j

    key = f"nc@{repeat}"
    if key not in _NC_CACHE:
        _NC_CACHE[key] = _build_nc(repeat=repeat)
    nc = _NC_CACHE[key]
    b2j.install_neuronx_cc_hook()

    in_maps = _host_prep(inputs)
    from concourse import mybir
    in_names, out_names, out_avals, zero_outs = [], [], [], []
    partition_name = nc.partition_id_tensor.name if nc.partition_id_tensor else None
    for alloc in nc.m.functions[0].allocations:
        if not isinstance(alloc, mybir.MemoryLocationSet):
            continue
        name = alloc.memorylocations[0].name
        if alloc.kind == "ExternalInput":
            if name != partition_name:
                in_names.append(name)
        elif alloc.kind == "ExternalOutput":
            out_names.append(name)
            shape = list(alloc.tensor_shape)
            np_dt = np.dtype(mybir.dt.np(alloc.dtype))
            out_avals.append(jax.core.ShapedArray(shape, np_dt))
            zero_outs.append(np.zeros(shape, np_dt))
    n_params = len(in_names)
    all_in_names = list(in_names) + out_names
    if partition_name is not None:
        all_in_names.append(partition_name)

    def _body(*args):
        operands = list(args)
        if partition_name is not None:
            operands.append(b2j.partition_id_tensor())
        outs = b2j._bass_exec_p.bind(
            *operands, out_avals=tuple(out_avals), in_names=tuple(all_in_names),
            out_names=tuple(out_names), lowering_input_output_aliases=(),
            sim_require_finite=True, sim_require_nnan=True, nc=nc)
        return tuple(outs)

    devices = jax.devices()[:B]
    mesh = Mesh(np.asarray(devices), ("core",))
    nio = n_params + len(out_names)
    fn = jax.jit(shard_map(_body, mesh=mesh,
                           in_specs=(PartitionSpec("core"),) * nio,
                           out_specs=(PartitionSpec("core"),) * len(out_names),
                           check_rep=False), keep_unused=True)
    concat_in = [np.concatenate([np.asarray(in_maps[c][n]) for c in range(B)], axis=0)
                 for n in in_names]
    concat_zeros = [np.zeros((B * z.shape[0], *z.shape[1:]), z.dtype) for z in zero_outs]
    from jax.sharding import NamedSharding
    sh = NamedSharding(mesh, PartitionSpec("core"))
    dev_in = [jax.device_put(x, sh) for x in concat_in + concat_zeros]
    _NC_CACHE[f"bench_fn@{repeat}"] = (fn, dev_in)
    outs = fn(*dev_in)
    jax.block_until_ready(outs)
    times = []
    for _ in range(iters):
        t0 = time.perf_counter()
        outs = fn(*dev_in)
        jax.block_until_ready(outs)
        times.append(time.perf_counter() - t0)
    return min(times), times


def benchmark_pipelined(inputs, n_lo=100, n_hi=200, repeat=1):
    """Marginal per-call time from pipelined async dispatches: amortizes the
    axon round-trip latency; returns (T(n_hi)-T(n_lo))/(n_hi-n_lo) seconds."""
    import time
    import jax
    # benchmark() must have been called first (compiles + stages buffers)
    if f"bench_fn@{repeat}" not in _NC_CACHE:
        benchmark(inputs, iters=1, repeat=repeat)
    fn, dev_in = _NC_CACHE[f"bench_fn@{repeat}"]
    ts = {}
    for n in (n_lo, n_hi):
        t0 = time.perf_counter()
        outs = [fn(*dev_in) for _ in range(n)]
        jax.block_until_ready(outs)
        ts[n] = time.perf_counter() - t0
    return (ts[n_hi] - ts[n_lo]) / (n_hi - n_lo)
